# revision 44
# baseline (speedup 1.0000x reference)
"""DTNetv0 forward kernel for 8 Trainium2 NeuronCores.

Computes, for x [B,128], W1 [511,128], b1 [511], W2 [512,1022],
leaf_actions [512] (32 leaves per each of 16 actions):

    h = x @ W1.T + b1
    z = [relu(h), relu(-h)]
    y = z @ W2.T
    pooled[b,a] = max over leaves l with action a of y[b,l]
    out = softmax(pooled, axis=-1)

Sharding: pure data parallelism — batch split 8 ways, weights replicated.

On-device layout keeps feature dims on SBUF partitions and batch on the
free dim so both matmuls contract over the partition dim without any
intermediate transposes:
    xT [128in, 512b] (PE transpose) -> hT [512nodes, 512b] (4 PSUM banks)
    -> zT [1024, 512b] (Relu(h+b1) on ACT; two relu(-h-b1) chunks on ACT
       via scale=-1, two as min(h+b1,0) on DVE against host-negated W2
       rows — identical math, balances the engines)
    -> yT [512leaves, 512b] (32 accumulating f32r matmuls)
Leaves are host-permuted so slot s holds a leaf of action s%16; the
segment max is then: DVE max across the four 128-row chunks (equal base
partitions), a PE transpose back to batch-major, and one strided DVE
reduce over the remaining 8 slots per action. Softmax runs batch-major:
Exp with per-partition bias=-max and accum_out for the denominator.

The per-tile tail (transpose-back, reduce, softmax, store) is emitted
one iteration behind the front half, so the scalar engine finishes the
z-chunks PE needs for matmul2 before it turns to the previous tile's
softmax.

Matmul operands use float32r: fp32 data processed at 1 cycle/row for
512-wide moving operands (plain fp32 runs at 4 cycles/row).
"""

import numpy as np

B, IN_DIM, N_NODES, N_LEAVES, N_ACTIONS = 131072, 128, 511, 512, 16
N_CORES = 8
B_SHARD = B // N_CORES          # 16384 rows per core
B_TILE = 512                    # batch columns per tile (one PSUM bank of fp32)
N_TILES = B_SHARD // B_TILE     # 32
NODES_P = 512                   # nodes padded 511 -> 512 (4 chunks of 128)
Z_DIM = 2 * NODES_P             # 1024 (8 chunks of 128)
DVE_ZHI = (0, 1)                # z_hi chunks produced on DVE via min-trick
# bf16 z/W2 for matmul2 was measured on HW at parity with f32r (the PE is
# row-rate-limited regardless of dtype) while costing 8x accuracy — keep f32r.
MM2_BF16 = False

_compiled = None  # traced+compiled Bass module cache (one per process)


def _build_nc(n_passes=1):
    import concourse.tile as tile
    from concourse import bacc, mybir
    from concourse.masks import make_identity
    from contextlib import ExitStack

    fp32 = mybir.dt.float32
    f32r = mybir.dt.float32r
    zdt = mybir.dt.bfloat16 if MM2_BF16 else f32r
    AF = mybir.ActivationFunctionType

    nc = bacc.Bacc()
    x_h = nc.declare_dram_parameter("x", [B_SHARD, IN_DIM], f32r, isOutput=False)
    w1t_h = nc.declare_dram_parameter("w1t", [IN_DIM, NODES_P], f32r, isOutput=False)
    b1c_h = nc.declare_dram_parameter("b1c", [128, 4], fp32, isOutput=False)
    nb1c_h = nc.declare_dram_parameter("nb1c", [128, 4], fp32, isOutput=False)
    w2t_h = nc.declare_dram_parameter("w2t", [128, 8, B_TILE], zdt, isOutput=False)
    out_h = nc.declare_dram_parameter("out", [B_SHARD, N_ACTIONS], fp32, isOutput=True)

    with tile.TileContext(nc) as tc, ExitStack() as ctx:
        consts = ctx.enter_context(tc.tile_pool(name="consts", bufs=1))
        xin = ctx.enter_context(tc.tile_pool(name="xin", bufs=3))
        xts = ctx.enter_context(tc.tile_pool(name="xts", bufs=2))
        zp = ctx.enter_context(tc.tile_pool(name="zp", bufs=3))
        sm = ctx.enter_context(tc.tile_pool(name="sm", bufs=2))
        psA = ctx.enter_context(tc.tile_pool(name="psA", bufs=2, space="PSUM"))
        psY = ctx.enter_context(tc.tile_pool(name="psY", bufs=5, space="PSUM"))

        def load_x(t):
            rows = slice(t * B_TILE, (t + 1) * B_TILE)
            x_sb = xin.tile([128, 4, IN_DIM], f32r, tag="x")
            nc.sync.dma_start(
                out=x_sb, in_=x_h[rows, :].rearrange("(s p) d -> p s d", p=128)
            )
            return x_sb

        identity = consts.tile([128, 128], fp32)
        make_identity(nc, identity)
        identity_r = consts.tile([128, 128], f32r)
        nc.vector.tensor_copy(identity_r, identity)
        # prefetch the first two x tiles before the (big) weight DMAs so the
        # first transposes are not queued behind them
        x_pre = [load_x(0), load_x(1)]
        b1_sb = consts.tile([128, 4], fp32)
        nc.sync.dma_start(out=b1_sb, in_=b1c_h[:, :])
        nb1_sb = consts.tile([128, 4], fp32)
        nc.sync.dma_start(out=nb1_sb, in_=nb1c_h[:, :])
        # weights are declared float32r in DRAM (host sends fp32 bits) and
        # DMA straight into f32r tiles — no staging/rounding copies. The 2MB
        # w2t load rides the Activation HWDGE queue so the x-loads (SP
        # queue) are not stuck behind it.
        w1t_sb = consts.tile([128, NODES_P], f32r)
        nc.sync.dma_start(out=w1t_sb, in_=w1t_h[:, :])
        w2t_sb = consts.tile([128, 8, B_TILE], zdt)
        nc.scalar.dma_start(out=w2t_sb, in_=w2t_h[:, :, :])

        def front_a(t, x_sb=None):
            rows = slice(t * B_TILE, (t + 1) * B_TILE)

            # ---- x tile (possibly prefetched) -> transpose to [in, batch] ----
            if x_sb is None:
                x_sb = load_x(t)
            xt_ps = psA.tile([128, 4, 128], f32r, tag="xt", bufs=1)
            for s in range(4):
                nc.tensor.transpose(xt_ps[:, s, :], x_sb[:, s, :], identity_r)
            xt_sb = xts.tile([128, 4, 128], f32r, tag="xt_sb")
            nc.vector.tensor_copy(xt_sb, xt_ps)
            xt_mm = xt_sb.rearrange("p s d -> p (s d)")

            # ---- linear1 + fused bias/relu into zT [128, 8, 512] ----
            z_sb = zp.tile([128, 8, B_TILE], zdt, tag="z")
            for c in range(4):
                h_ps = psA.tile([128, B_TILE], fp32, tag="h")
                nc.tensor.matmul(
                    h_ps,
                    lhsT=w1t_sb[:, c * 128 : (c + 1) * 128],
                    rhs=xt_mm,
                    start=True,
                    stop=True,
                )
                nc.scalar.activation(
                    out=z_sb[:, c, :], in_=h_ps, func=AF.Relu,
                    bias=b1_sb[:, c : c + 1], scale=1.0,
                )
                if c in DVE_ZHI:
                    # min(h+b1, 0) = -relu(-h-b1); W2 rows for this chunk
                    # are negated host-side
                    nc.vector.tensor_scalar(
                        out=z_sb[:, 4 + c, :], in0=h_ps,
                        scalar1=b1_sb[:, c : c + 1], scalar2=0.0,
                        op0=mybir.AluOpType.add, op1=mybir.AluOpType.min,
                    )
                else:
                    nc.scalar.activation(
                        out=z_sb[:, 4 + c, :], in_=h_ps, func=AF.Relu,
                        bias=nb1_sb[:, c : c + 1], scale=-1.0,
                    )

            return rows, z_sb

        def front_b(rows, z_sb):
            # ---- linear2, batch-major: y_s [128 batch-sub, 512 leaves] ----
            # z is the stationary operand and W2T the moving one, so y comes
            # out batch-major and the segment max is a single strided
            # free-dim reduce straight off each PSUM bank — no
            # transpose-back, no partition folds.
            pl = sm.tile([128, 4, N_ACTIONS], fp32, tag="pl")
            for s in range(4):
                y_ps = psY.tile([128, B_TILE], fp32, tag="y")
                for k in range(8):
                    nc.tensor.matmul(
                        y_ps,
                        lhsT=z_sb[:, k, s * 128 : (s + 1) * 128],
                        rhs=w2t_sb[:, k, :],
                        start=(k == 0),
                        stop=(k == 7),
                    )
                nc.vector.tensor_reduce(
                    out=pl[:, s, :],
                    in_=y_ps.rearrange("p (j a) -> p a j", a=N_ACTIONS),
                    axis=mybir.AxisListType.X,
                    op=mybir.AluOpType.max,
                )
            return rows, pl

        def back(rows, pl):
            # ---- softmax, batch-major [128, 4, 16] ----
            negmx = sm.tile([128, 4], fp32, tag="negmx")
            nc.vector.tensor_reduce(
                out=negmx, in_=pl, axis=mybir.AxisListType.X,
                op=mybir.AluOpType.max, negate=True,
            )
            e = sm.tile([128, 4, N_ACTIONS], fp32, tag="e")
            ssum = sm.tile([128, 4], fp32, tag="ssum")
            for s in range(4):
                nc.scalar.activation(
                    out=e[:, s, :], in_=pl[:, s, :], func=AF.Exp,
                    bias=negmx[:, s : s + 1], scale=1.0,
                    accum_out=ssum[:, s : s + 1],
                )
            rcp = sm.tile([128, 4], fp32, tag="rcp")
            nc.vector.reciprocal(rcp, ssum)
            o = sm.tile([128, 4, N_ACTIONS], fp32, tag="o")
            for s in range(4):
                nc.vector.tensor_scalar_mul(o[:, s, :], e[:, s, :], rcp[:, s : s + 1])

            nc.sync.dma_start(
                out=out_h[rows, :].rearrange("(s p) a -> p s a", p=128), in_=o
            )

        # 3-deep software pipeline: front_a (x -> z) runs two tiles ahead
        # of front_b (mm2 + pooled reduce), so the scalar-engine z chunks are
        # ready long before their matmul2; back trails one tile behind.
        total = N_TILES * n_passes
        fa = [front_a(0, x_pre[0])]
        if total > 1:
            fa.append(front_a(1, x_pre[1]))
        pending = None
        for i in range(total):
            cur = front_b(*fa.pop(0))
            if i + 2 < total:
                fa.append(front_a((i + 2) % N_TILES))
            if pending is not None:
                back(*pending)
            pending = cur
        back(*pending)

    nc.compile()
    return nc


def _prep_weights(W1, b1, W2, leaf_actions):
    """Host-side weight prep: pad/transpose W1, pad W2 and permute leaves so
    slot s holds a leaf of action s % 16 (round-robin over each group).
    W2 rows feeding the DVE-produced z_hi chunks (min-trick) are negated."""
    w1t = np.zeros((IN_DIM, NODES_P), np.float32)
    w1t[:, :N_NODES] = np.asarray(W1, np.float32).T
    b1c = np.zeros((4, 128), np.float32)
    b1c.reshape(-1)[:N_NODES] = np.asarray(b1, np.float32)
    b1c = np.ascontiguousarray(b1c.T)          # [128, 4]
    nb1c = np.ascontiguousarray(-b1c)

    la = np.asarray(leaf_actions).astype(np.int64)
    perm = np.empty(N_LEAVES, np.int64)
    per_action = N_LEAVES // N_ACTIONS
    for a in range(N_ACTIONS):
        (grp,) = np.nonzero(la == a)
        assert len(grp) == per_action, "kernel assumes 32 leaves per action"
        perm[a + N_ACTIONS * np.arange(per_action)] = grp

    W2 = np.asarray(W2, np.float32)[perm]       # [512, 1022] leaf-permuted
    w2t = np.zeros((Z_DIM, N_LEAVES), np.float32)
    w2t[:N_NODES, :] = W2[:, :N_NODES].T        # relu(h) half
    w2t[NODES_P : NODES_P + N_NODES, :] = W2[:, N_NODES:].T  # relu(-h) half
    for c in DVE_ZHI:                           # min-trick chunks: z negated
        w2t[NODES_P + c * 128 : NODES_P + (c + 1) * 128, :] *= -1.0
    w2t = np.ascontiguousarray(
        w2t.reshape(8, 128, N_LEAVES).transpose(1, 0, 2)
    )                                           # [128, 8, 512]
    if MM2_BF16:
        import ml_dtypes
        w2t = w2t.astype(ml_dtypes.bfloat16)
    return w1t, b1c, nb1c, w2t


_runner = None  # (jitted shard_map fn, in_names, zeros) — persists across calls


def _make_runner(nc):
    """Jitted shard_map wrapper around the bass_exec custom call (mirrors
    bass2jax.run_bass_via_pjrt's multi-core path, but reusable across calls
    so the NEFF is compiled once per process)."""
    import jax
    import numpy as _np
    from jax.sharding import Mesh, PartitionSpec, NamedSharding
    from jax.experimental.shard_map import shard_map
    from concourse import bass2jax, mybir

    bass2jax.install_neuronx_cc_hook()
    partition_name = nc.partition_id_tensor.name if nc.partition_id_tensor else None
    in_names, out_names, out_avals, zero_shapes = [], [], [], []
    for alloc in nc.m.functions[0].allocations:
        if not isinstance(alloc, mybir.MemoryLocationSet):
            continue
        name = alloc.memorylocations[0].name
        if alloc.kind == "ExternalInput":
            if name != partition_name:
                in_names.append(name)
        elif alloc.kind == "ExternalOutput":
            shape = tuple(alloc.tensor_shape)
            dtype = mybir.dt.np(alloc.dtype)
            out_names.append(name)
            out_avals.append(jax.core.ShapedArray(shape, dtype))
            zero_shapes.append((shape, dtype))
    n_params = len(in_names)
    all_in_names = in_names + out_names + ([partition_name] if partition_name else [])

    def _body(*args):
        operands = list(args)
        if partition_name is not None:
            operands.append(bass2jax.partition_id_tensor())
        return tuple(bass2jax._bass_exec_p.bind(
            *operands, out_avals=tuple(out_avals), in_names=tuple(all_in_names),
            out_names=tuple(out_names), lowering_input_output_aliases=(),
            sim_require_finite=True, sim_require_nnan=True, nc=nc))

    mesh = Mesh(_np.asarray(jax.devices()[:N_CORES]), ("core",))
    spec = PartitionSpec("core")
    n_outs = len(out_names)
    fn = jax.jit(
        shard_map(_body, mesh=mesh, in_specs=(spec,) * (n_params + n_outs),
                  out_specs=(spec,) * n_outs, check_rep=False),
        keep_unused=True)
    sh = NamedSharding(mesh, spec)
    zeros = tuple(
        jax.device_put(_np.zeros((N_CORES * s[0], *s[1:]), d), sh)
        for s, d in zero_shapes)
    return fn, in_names, sh, zeros


def kernel(x, W1, b1, W2, leaf_actions):
    global _compiled, _runner
    import jax

    x = np.ascontiguousarray(np.asarray(x, np.float32))
    assert x.shape == (B, IN_DIM)
    w1t, b1c, nb1c, w2t = _prep_weights(W1, b1, W2, leaf_actions)

    if _compiled is None:
        _compiled = _build_nc()
    if _runner is None:
        _runner = _make_runner(_compiled)
    fn, in_names, sh, zeros = _runner

    full = {"x": x, "w1t": np.concatenate([w1t] * N_CORES, axis=0),
            "b1c": np.concatenate([b1c] * N_CORES, axis=0),
            "nb1c": np.concatenate([nb1c] * N_CORES, axis=0),
            "w2t": np.concatenate([w2t] * N_CORES, axis=0)}
    dev_in = [jax.device_put(full[nm], sh) for nm in in_names]
    out = fn(*dev_in, *zeros)
    return np.asarray(out[0])


# revision 45
# speedup vs baseline: 1.5501x; 1.5501x over previous
"""DTNetv0 forward kernel for 8 Trainium2 NeuronCores.

Computes, for x [B,128], W1 [511,128], b1 [511], W2 [512,1022],
leaf_actions [512] (32 leaves per each of 16 actions):

    h = x @ W1.T + b1
    z = [relu(h), relu(-h)]
    y = z @ W2.T
    pooled[b,a] = max over leaves l with action a of y[b,l]
    out = softmax(pooled, axis=-1)

Sharding: pure data parallelism — batch split 8 ways, weights replicated.

Per 512-row batch tile, on device:
    xT [128in, 512b]   PE transpose of the DMA'd x tile
    hT [512nodes,512b] linear1: 4 f32r matmuls (W1T stationary, xT moving)
    zT [1024, 512b]    Relu(h+b1) on ACT; two relu(-h-b1) chunks on ACT via
                       scale=-1, two as min(h+b1,0) on DVE against
                       host-negated W2 rows (identical math, balances the
                       engines)
    y  [128b, 512lv]   linear2 BATCH-MAJOR: per 128-batch subtile, 8
                       accumulating f32r matmuls with the z chunk as the
                       STATIONARY operand and W2T as the moving one, so y
                       lands batch-major in PSUM
    pooled [128b, 16]  leaves are host-permuted so slot s holds a leaf of
                       action s%16; the whole segment max is ONE strided DVE
                       reduce straight off each PSUM bank (no transposes, no
                       partition folds)
    out                softmax batch-major: Exp with per-partition bias=-max
                       and accum_out for the denominator, reciprocal, scale

Three-stage software pipeline in emission order: front_a (x -> z) runs two
tiles ahead of front_b (matmul2 + pooled reduce), and the softmax tail
trails one tile behind, so the z chunks PE needs are always ready and the
cost-model steady state is PE-gap-free (94% tensor-engine busy).

Matmul operands use float32r: fp32 data processed at 1 cycle/row for
512-wide moving operands (plain fp32 runs at 4 cycles/row). bf16 was
measured on HW at parity with f32r (the PE is row-rate-limited regardless
of dtype) while costing 8x accuracy, so f32r stays.
"""

import numpy as np

B, IN_DIM, N_NODES, N_LEAVES, N_ACTIONS = 131072, 128, 511, 512, 16
N_CORES = 8
B_SHARD = B // N_CORES          # 16384 rows per core
B_TILE = 512                    # batch columns per tile (one PSUM bank of fp32)
N_TILES = B_SHARD // B_TILE     # 32
NODES_P = 512                   # nodes padded 511 -> 512 (4 chunks of 128)
Z_DIM = 2 * NODES_P             # 1024 (8 chunks of 128)
DVE_ZHI = (0, 1)                # z_hi chunks produced on DVE via min-trick
# bf16 z/W2 for matmul2 was measured on HW at parity with f32r (the PE is
# row-rate-limited regardless of dtype) while costing 8x accuracy — keep f32r.
MM2_BF16 = False

_compiled = None  # traced+compiled Bass module cache (one per process)


def _build_nc(n_passes=1):
    import concourse.tile as tile
    from concourse import bacc, mybir
    from concourse.masks import make_identity
    from contextlib import ExitStack

    fp32 = mybir.dt.float32
    f32r = mybir.dt.float32r
    zdt = mybir.dt.bfloat16 if MM2_BF16 else f32r
    AF = mybir.ActivationFunctionType

    nc = bacc.Bacc()
    x_h = nc.declare_dram_parameter("x", [B_SHARD, IN_DIM], f32r, isOutput=False)
    w1t_h = nc.declare_dram_parameter("w1t", [IN_DIM, NODES_P], f32r, isOutput=False)
    b1c_h = nc.declare_dram_parameter("b1c", [128, 4], fp32, isOutput=False)
    nb1c_h = nc.declare_dram_parameter("nb1c", [128, 4], fp32, isOutput=False)
    w2t_h = nc.declare_dram_parameter("w2t", [128, 8, B_TILE], zdt, isOutput=False)
    out_h = nc.declare_dram_parameter("out", [B_SHARD, N_ACTIONS], fp32, isOutput=True)

    with tile.TileContext(nc) as tc, ExitStack() as ctx:
        consts = ctx.enter_context(tc.tile_pool(name="consts", bufs=1))
        xin = ctx.enter_context(tc.tile_pool(name="xin", bufs=3))
        xts = ctx.enter_context(tc.tile_pool(name="xts", bufs=2))
        zp = ctx.enter_context(tc.tile_pool(name="zp", bufs=3))
        sm = ctx.enter_context(tc.tile_pool(name="sm", bufs=2))
        psA = ctx.enter_context(tc.tile_pool(name="psA", bufs=2, space="PSUM"))
        psY = ctx.enter_context(tc.tile_pool(name="psY", bufs=5, space="PSUM"))

        def load_x(t):
            rows = slice(t * B_TILE, (t + 1) * B_TILE)
            x_sb = xin.tile([128, 4, IN_DIM], f32r, tag="x")
            nc.sync.dma_start(
                out=x_sb, in_=x_h[rows, :].rearrange("(s p) d -> p s d", p=128)
            )
            return x_sb

        identity = consts.tile([128, 128], fp32)
        make_identity(nc, identity)
        identity_r = consts.tile([128, 128], f32r)
        nc.vector.tensor_copy(identity_r, identity)
        # prefetch the first two x tiles before the (big) weight DMAs so the
        # first transposes are not queued behind them
        x_pre = [load_x(0), load_x(1)]
        b1_sb = consts.tile([128, 4], fp32)
        nc.sync.dma_start(out=b1_sb, in_=b1c_h[:, :])
        nb1_sb = consts.tile([128, 4], fp32)
        nc.sync.dma_start(out=nb1_sb, in_=nb1c_h[:, :])
        # weights are declared float32r in DRAM (host sends fp32 bits) and
        # DMA straight into f32r tiles — no staging/rounding copies. The 2MB
        # w2t load rides the Activation HWDGE queue so the x-loads (SP
        # queue) are not stuck behind it.
        w1t_sb = consts.tile([128, NODES_P], f32r)
        nc.sync.dma_start(out=w1t_sb, in_=w1t_h[:, :])
        w2t_sb = consts.tile([128, 8, B_TILE], zdt)
        nc.scalar.dma_start(out=w2t_sb, in_=w2t_h[:, :, :])

        def front_a(t, x_sb=None):
            rows = slice(t * B_TILE, (t + 1) * B_TILE)

            # ---- x tile (possibly prefetched) -> transpose to [in, batch] ----
            if x_sb is None:
                x_sb = load_x(t)
            xt_ps = psA.tile([128, 4, 128], f32r, tag="xt", bufs=1)
            for s in range(4):
                nc.tensor.transpose(xt_ps[:, s, :], x_sb[:, s, :], identity_r)
            xt_sb = xts.tile([128, 4, 128], f32r, tag="xt_sb")
            nc.vector.tensor_copy(xt_sb, xt_ps)
            xt_mm = xt_sb.rearrange("p s d -> p (s d)")

            # ---- linear1 + fused bias/relu into zT [128, 8, 512] ----
            z_sb = zp.tile([128, 8, B_TILE], zdt, tag="z")
            for c in range(4):
                h_ps = psA.tile([128, B_TILE], fp32, tag="h")
                nc.tensor.matmul(
                    h_ps,
                    lhsT=w1t_sb[:, c * 128 : (c + 1) * 128],
                    rhs=xt_mm,
                    start=True,
                    stop=True,
                )
                nc.scalar.activation(
                    out=z_sb[:, c, :], in_=h_ps, func=AF.Relu,
                    bias=b1_sb[:, c : c + 1], scale=1.0,
                )
                if c in DVE_ZHI:
                    # min(h+b1, 0) = -relu(-h-b1); W2 rows for this chunk
                    # are negated host-side
                    nc.vector.tensor_scalar(
                        out=z_sb[:, 4 + c, :], in0=h_ps,
                        scalar1=b1_sb[:, c : c + 1], scalar2=0.0,
                        op0=mybir.AluOpType.add, op1=mybir.AluOpType.min,
                    )
                else:
                    nc.scalar.activation(
                        out=z_sb[:, 4 + c, :], in_=h_ps, func=AF.Relu,
                        bias=nb1_sb[:, c : c + 1], scale=-1.0,
                    )

            return rows, z_sb

        def front_b(rows, z_sb):
            # ---- linear2, batch-major: y_s [128 batch-sub, 512 leaves] ----
            # z is the stationary operand and W2T the moving one, so y comes
            # out batch-major and the segment max is a single strided
            # free-dim reduce straight off each PSUM bank — no
            # transpose-back, no partition folds.
            pl = sm.tile([128, 4, N_ACTIONS], fp32, tag="pl")
            for s in range(4):
                y_ps = psY.tile([128, B_TILE], fp32, tag="y")
                for k in range(8):
                    nc.tensor.matmul(
                        y_ps,
                        lhsT=z_sb[:, k, s * 128 : (s + 1) * 128],
                        rhs=w2t_sb[:, k, :],
                        start=(k == 0),
                        stop=(k == 7),
                    )
                nc.vector.tensor_reduce(
                    out=pl[:, s, :],
                    in_=y_ps.rearrange("p (j a) -> p a j", a=N_ACTIONS),
                    axis=mybir.AxisListType.X,
                    op=mybir.AluOpType.max,
                )
            return rows, pl

        def back(rows, pl):
            # ---- softmax, batch-major [128, 4, 16] ----
            negmx = sm.tile([128, 4], fp32, tag="negmx")
            nc.vector.tensor_reduce(
                out=negmx, in_=pl, axis=mybir.AxisListType.X,
                op=mybir.AluOpType.max, negate=True,
            )
            e = sm.tile([128, 4, N_ACTIONS], fp32, tag="e")
            ssum = sm.tile([128, 4], fp32, tag="ssum")
            for s in range(4):
                nc.scalar.activation(
                    out=e[:, s, :], in_=pl[:, s, :], func=AF.Exp,
                    bias=negmx[:, s : s + 1], scale=1.0,
                    accum_out=ssum[:, s : s + 1],
                )
            rcp = sm.tile([128, 4], fp32, tag="rcp")
            nc.vector.reciprocal(rcp, ssum)
            o = sm.tile([128, 4, N_ACTIONS], fp32, tag="o")
            for s in range(4):
                nc.vector.tensor_scalar_mul(o[:, s, :], e[:, s, :], rcp[:, s : s + 1])

            nc.sync.dma_start(
                out=out_h[rows, :].rearrange("(s p) a -> p s a", p=128), in_=o
            )

        # 3-deep software pipeline: front_a (x -> z) runs two tiles ahead
        # of front_b (mm2 + pooled reduce), so the scalar-engine z chunks are
        # ready long before their matmul2; back trails one tile behind.
        total = N_TILES * n_passes
        fa = [front_a(0, x_pre[0])]
        if total > 1:
            fa.append(front_a(1, x_pre[1]))
        pending = None
        for i in range(total):
            cur = front_b(*fa.pop(0))
            if i + 2 < total:
                fa.append(front_a((i + 2) % N_TILES))
            if pending is not None:
                back(*pending)
            pending = cur
        back(*pending)

    nc.compile()
    return nc


def _prep_weights(W1, b1, W2, leaf_actions):
    """Host-side weight prep: pad/transpose W1, pad W2 and permute leaves so
    slot s holds a leaf of action s % 16 (round-robin over each group).
    W2 rows feeding the DVE-produced z_hi chunks (min-trick) are negated."""
    w1t = np.zeros((IN_DIM, NODES_P), np.float32)
    w1t[:, :N_NODES] = np.asarray(W1, np.float32).T
    b1c = np.zeros((4, 128), np.float32)
    b1c.reshape(-1)[:N_NODES] = np.asarray(b1, np.float32)
    b1c = np.ascontiguousarray(b1c.T)          # [128, 4]
    nb1c = np.ascontiguousarray(-b1c)

    la = np.asarray(leaf_actions).astype(np.int64)
    perm = np.empty(N_LEAVES, np.int64)
    per_action = N_LEAVES // N_ACTIONS
    for a in range(N_ACTIONS):
        (grp,) = np.nonzero(la == a)
        assert len(grp) == per_action, "kernel assumes 32 leaves per action"
        perm[a + N_ACTIONS * np.arange(per_action)] = grp

    W2 = np.asarray(W2, np.float32)[perm]       # [512, 1022] leaf-permuted
    w2t = np.zeros((Z_DIM, N_LEAVES), np.float32)
    w2t[:N_NODES, :] = W2[:, :N_NODES].T        # relu(h) half
    w2t[NODES_P : NODES_P + N_NODES, :] = W2[:, N_NODES:].T  # relu(-h) half
    for c in DVE_ZHI:                           # min-trick chunks: z negated
        w2t[NODES_P + c * 128 : NODES_P + (c + 1) * 128, :] *= -1.0
    w2t = np.ascontiguousarray(
        w2t.reshape(8, 128, N_LEAVES).transpose(1, 0, 2)
    )                                           # [128, 8, 512]
    if MM2_BF16:
        import ml_dtypes
        w2t = w2t.astype(ml_dtypes.bfloat16)
    return w1t, b1c, nb1c, w2t


_runner = None  # (jitted shard_map fn, in_names, zeros) — persists across calls


def _make_runner(nc):
    """Jitted shard_map wrapper around the bass_exec custom call (mirrors
    bass2jax.run_bass_via_pjrt's multi-core path, but reusable across calls
    so the NEFF is compiled once per process)."""
    import jax
    import numpy as _np
    from jax.sharding import Mesh, PartitionSpec, NamedSharding
    from jax.experimental.shard_map import shard_map
    from concourse import bass2jax, mybir

    bass2jax.install_neuronx_cc_hook()
    partition_name = nc.partition_id_tensor.name if nc.partition_id_tensor else None
    in_names, out_names, out_avals, zero_shapes = [], [], [], []
    for alloc in nc.m.functions[0].allocations:
        if not isinstance(alloc, mybir.MemoryLocationSet):
            continue
        name = alloc.memorylocations[0].name
        if alloc.kind == "ExternalInput":
            if name != partition_name:
                in_names.append(name)
        elif alloc.kind == "ExternalOutput":
            shape = tuple(alloc.tensor_shape)
            dtype = mybir.dt.np(alloc.dtype)
            out_names.append(name)
            out_avals.append(jax.core.ShapedArray(shape, dtype))
            zero_shapes.append((shape, dtype))
    n_params = len(in_names)
    all_in_names = in_names + out_names + ([partition_name] if partition_name else [])

    def _body(*args):
        operands = list(args)
        if partition_name is not None:
            operands.append(bass2jax.partition_id_tensor())
        return tuple(bass2jax._bass_exec_p.bind(
            *operands, out_avals=tuple(out_avals), in_names=tuple(all_in_names),
            out_names=tuple(out_names), lowering_input_output_aliases=(),
            sim_require_finite=True, sim_require_nnan=True, nc=nc))

    mesh = Mesh(_np.asarray(jax.devices()[:N_CORES]), ("core",))
    spec = PartitionSpec("core")
    n_outs = len(out_names)
    fn = jax.jit(
        shard_map(_body, mesh=mesh, in_specs=(spec,) * (n_params + n_outs),
                  out_specs=(spec,) * n_outs, check_rep=False),
        keep_unused=True)
    sh = NamedSharding(mesh, spec)
    zeros = tuple(
        jax.device_put(_np.zeros((N_CORES * s[0], *s[1:]), d), sh)
        for s, d in zero_shapes)
    return fn, in_names, sh, zeros


def kernel(x, W1, b1, W2, leaf_actions):
    global _compiled, _runner
    import jax

    x = np.ascontiguousarray(np.asarray(x, np.float32))
    assert x.shape == (B, IN_DIM)
    w1t, b1c, nb1c, w2t = _prep_weights(W1, b1, W2, leaf_actions)

    if _compiled is None:
        _compiled = _build_nc()
    if _runner is None:
        _runner = _make_runner(_compiled)
    fn, in_names, sh, zeros = _runner

    full = {"x": x, "w1t": np.concatenate([w1t] * N_CORES, axis=0),
            "b1c": np.concatenate([b1c] * N_CORES, axis=0),
            "nb1c": np.concatenate([nb1c] * N_CORES, axis=0),
            "w2t": np.concatenate([w2t] * N_CORES, axis=0)}
    dev_in = [jax.device_put(full[nm], sh) for nm in in_names]
    out = fn(*dev_in, *zeros)
    return np.asarray(out[0])
